# revision 12
# baseline (speedup 1.0000x reference)
"""Trainium2 Bass kernel for nn_ConvLayer_13967233646751 (gnn_message_passing).

v6 (baseline 161us; v5 171us — scatter-DMA supply-bound):
  The v5 trace showed the p-major -> c-major transpose of the rotated geo
  (9216 64-byte descriptors per batch through DRAM) costing 4-13us of DMA
  per 2-q-group and serializing the whole supply chain (gpsimd busy to
  t=133us).  v6 eliminates the scatter entirely:

  - rotation output alq is laid out [p, s, q_lo, c(32, zero-padded)] fp16
    per half-batch, DMA'd to DRAM fully contiguously (128 x 8KB
    descriptors), and read back with the hardware XBAR DMA transpose
    ([4096, 128] -> [128, 4096]), landing as partition rows 32*q_lo + c.
  - GEMM1 is split: feats (K=64, partitions 0:64) + geo (K=32 at base
    partition 32*q_lo, weights replicated at all four 32-row offsets with
    zero rows for the 23 pad channels), accumulating into the same PSUM.
  - geo input is loaded fp16 in [p, q, s, j, t] layout (no cast DMA).
  - R phase: unitization via ACT exp(-0.5*ln(ss+1e-12)) (== 1/(|v|+1e-8)
    to fp16 precision); xyz sums are elementwise adds.  b1's R phase runs
    on gpsimd+ACT only, so it never blocks the DVE queue.
  - rotation: b0-half0 on DVE (idle at startup), the rest on gpsimd.
  - redundant LDWEIGHTS dropped on the 2nd matmul of same-weight pairs.
  - PE warmup burst gated on rot-b0h0 so HAM is at 8/8 for chunk 0.
"""

import numpy as np
from contextlib import ExitStack

import concourse.bass as bass
import concourse.tile as tile
from concourse import bacc
from concourse import mybir
from concourse.bass_utils import run_bass_kernel_spmd

F32 = mybir.dt.float32
F16 = mybir.dt.float16
AX = mybir.AxisListType
OP = mybir.AluOpType
AF = mybir.ActivationFunctionType

EPS = 1e-8
B, C, P, S = 16, 76, 1024, 32
NCORES = 8
BL = B // NCORES
NQ = P // 128
NB = BL * NQ
NCH = 32

GEO_W1_COLS = [67, 0, 70, 68, 1, 71, 69, 2, 72]

SKIP_DUP_LDW = True


def build_program():
    nc = bacc.Bacc()

    feats_d = nc.dram_tensor("feats", [BL, 64, NQ, 4096], F16, kind="ExternalInput")
    geo_d = nc.dram_tensor("geo", [BL, 128, NQ, S, 3, 3], F16, kind="ExternalInput")
    norm_d = nc.dram_tensor("normp", [128, 3, BL, NQ], F32, kind="ExternalInput")
    w1f_d = nc.dram_tensor("w1f", [64, 128], F16, kind="ExternalInput")
    w1g_d = nc.dram_tensor("w1g", [128, 128], F16, kind="ExternalInput")
    w2T_d = nc.dram_tensor("w2T", [128, 128], F16, kind="ExternalInput")
    b1_d = nc.dram_tensor("b1c", [128, 1], F32, kind="ExternalInput")
    b2_d = nc.dram_tensor("b2c", [128, 1], F32, kind="ExternalInput")
    geoT_d = nc.dram_tensor("geot", [BL, 2, 128, S, 4, 32], F16, kind="ExternalInput")
    outp_d = nc.dram_tensor("outp", [BL, 128, P], F16, kind="ExternalOutput")
    outa_d = nc.dram_tensor("outa", [128, 3, BL, NQ], F32, kind="ExternalOutput")

    with tile.TileContext(nc) as tc, ExitStack() as ctx:
        cpool = ctx.enter_context(tc.tile_pool(name="const", bufs=1))
        geo_pool = ctx.enter_context(tc.tile_pool(name="geo", bufs=2))
        rpool = ctx.enter_context(tc.tile_pool(name="rphase", bufs=1))
        al_pool = ctx.enter_context(tc.tile_pool(name="aligned", bufs=1))
        tmp_pool = ctx.enter_context(tc.tile_pool(name="rtmp", bufs=2))
        xt_pool = ctx.enter_context(tc.tile_pool(name="xt", bufs=1))
        gt_pool = ctx.enter_context(tc.tile_pool(name="gt", bufs=3))
        h1_pool = ctx.enter_context(tc.tile_pool(name="h1", bufs=3))
        pb_pool = ctx.enter_context(tc.tile_pool(name="pooled", bufs=1))
        ps1_pool = ctx.enter_context(tc.tile_pool(name="ps1", bufs=2, space="PSUM"))
        ps2_pool = ctx.enter_context(tc.tile_pool(name="ps2", bufs=2, space="PSUM"))

        # ---- constants on sync HWDGE ----
        norm_pt = cpool.tile([128, 3, NB], F32)
        nc.sync.dma_start(out=norm_pt[:], in_=norm_d[:, :, :, :].rearrange("p x b q -> p x (b q)"))
        b1t = cpool.tile([128, 1], F32)
        nc.sync.dma_start(out=b1t[:], in_=b1_d[:, :])
        b2t = cpool.tile([128, 1], F32)
        nc.sync.dma_start(out=b2t[:], in_=b2_d[:, :])
        w1f = cpool.tile([64, 128], F16)
        nc.sync.dma_start(out=w1f[:], in_=w1f_d[:, :])
        w1g32 = cpool.tile([128, 128], F16)
        nc.sync.dma_start(out=w1g32[:], in_=w1g_d[:, :])
        w2T = cpool.tile([128, 128], F16)
        nc.sync.dma_start(out=w2T[:], in_=w2T_d[:, :])

        geo_pt = {}
        featsT = {}
        gts = {}
        for _b in range(BL):
            featsT[_b] = xt_pool.tile([64, NQ, 4096], F16, name=f"ft_{_b}")

        def emit_geo_load(b):
            g = geo_pool.tile([128, NQ, S, 3, 3], F16, tag="geo")
            nc.gpsimd.dma_start(out=g[:], in_=geo_d[b])
            geo_pt[b] = g

        def emit_feats_load(b, q0, nq):
            nc.gpsimd.dma_start(
                out=featsT[b][:, q0:q0 + nq].rearrange("c q f -> c (q f)"),
                in_=feats_d[b, :, q0:q0 + nq].rearrange("c q f -> c (q f)"))

        # ---------- R phase tiles ----------
        na = rpool.tile([128, 3, 2, NB], F32)
        sq2 = rpool.tile([128, 3, 2, NB], F32)
        ss2 = rpool.tile([128, 2, NB], F32)
        inv2 = rpool.tile([128, 2, NB], F32)
        u2 = rpool.tile([128, 3, 2, NB], F32)
        dot = rpool.tile([128, NB], F32)
        xraw = rpool.tile([128, 3, NB], F32)
        sqx = rpool.tile([128, 3, NB], F32)
        ssx = rpool.tile([128, NB], F32)
        nrmx = rpool.tile([128, NB], F32)
        invx = rpool.tile([128, NB], F32)
        x_u = rpool.tile([128, 3, NB], F32)
        yax = rpool.tile([128, 3, NB], F32)
        tmp3 = rpool.tile([128, 3, NB], F32)
        zero = rpool.tile([128, NB], F32)
        epsb = rpool.tile([128, 1], F32)
        outa_sb = cpool.tile([128, 3, BL, NQ], F32)

        def emit_azi(b):
            """The only real reduction (mean over s) — on DVE, early."""
            B_ = slice(b * NQ, (b + 1) * NQ)
            nc.vector.reduce_sum(
                out=na[:, :, 1, B_].transpose([0, 2, 1]),
                in_=geo_pt[b][:, :, 1:S, :, 1].transpose([0, 1, 3, 2]), axis=AX.X)

        def emit_rphase(b, eng):
            """R phase for one batch; elementwise on `eng`, unitization via
            ACT exp(-0.5*ln(ss+1e-12)).  No DVE deps after emit_azi."""
            B_ = slice(b * NQ, (b + 1) * NQ)
            if b == 0:
                eng.memset(zero[:], 0.0)
                eng.memset(epsb[:], 1e-12)
            eng.tensor_copy(out=na[:, :, 0, B_], in_=norm_pt[:, :, B_])
            eng.tensor_scalar_mul(out=na[:, :, 1, B_], in0=na[:, :, 1, B_],
                                  scalar1=1.0 / 31.0)
            nab = na[:, :, :, B_]
            eng.tensor_tensor(out=sq2[:, :, :, B_], in0=nab, in1=nab, op=OP.mult)
            eng.tensor_tensor(out=ss2[:, :, B_], in0=sq2[:, 0, :, B_],
                              in1=sq2[:, 1, :, B_], op=OP.add)
            eng.tensor_tensor(out=ss2[:, :, B_], in0=ss2[:, :, B_],
                              in1=sq2[:, 2, :, B_], op=OP.add)
            nc.scalar.activation(inv2[:, :, B_], ss2[:, :, B_], AF.Ln,
                                 bias=epsb[:, 0:1])
            nc.scalar.activation(inv2[:, :, B_], inv2[:, :, B_], AF.Exp,
                                 scale=-0.5)
            inv_b = inv2[:, :, B_].unsqueeze(1).broadcast_to([128, 3, 2, NQ])
            eng.tensor_tensor(out=u2[:, :, :, B_], in0=nab, in1=inv_b,
                              op=OP.mult)
            n_u = u2[:, :, 0, B_]
            a_u = u2[:, :, 1, B_]

            eng.tensor_tensor(out=tmp3[:, :, B_], in0=a_u, in1=n_u, op=OP.mult)
            eng.tensor_tensor(out=dot[:, B_], in0=tmp3[:, 0, B_],
                              in1=tmp3[:, 1, B_], op=OP.add)
            eng.tensor_tensor(out=dot[:, B_], in0=dot[:, B_],
                              in1=tmp3[:, 2, B_], op=OP.add)

            dot_b = dot[:, B_].unsqueeze(1).broadcast_to([128, 3, NQ])
            eng.tensor_tensor(out=xraw[:, :, B_], in0=dot_b, in1=n_u, op=OP.mult)
            eng.tensor_tensor(out=xraw[:, :, B_], in0=a_u, in1=xraw[:, :, B_],
                              op=OP.subtract)
            eng.tensor_tensor(out=sqx[:, :, B_], in0=xraw[:, :, B_],
                              in1=xraw[:, :, B_], op=OP.mult)
            eng.tensor_tensor(out=ssx[:, B_], in0=sqx[:, 0, B_],
                              in1=sqx[:, 1, B_], op=OP.add)
            eng.tensor_tensor(out=ssx[:, B_], in0=ssx[:, B_],
                              in1=sqx[:, 2, B_], op=OP.add)
            nc.scalar.activation(invx[:, B_], ssx[:, B_], AF.Ln,
                                 bias=epsb[:, 0:1])
            nc.scalar.activation(invx[:, B_], invx[:, B_], AF.Exp, scale=-0.5)
            invx_b = invx[:, B_].unsqueeze(1).broadcast_to([128, 3, NQ])
            eng.tensor_tensor(out=x_u[:, :, B_], in0=xraw[:, :, B_], in1=invx_b,
                              op=OP.mult)
            # |xraw| = ssx * (1/|xraw|), for the svec dir row
            eng.tensor_tensor(out=nrmx[:, B_], in0=ssx[:, B_], in1=invx[:, B_],
                              op=OP.mult)

            for x_ in range(3):
                i1, i2 = (x_ + 1) % 3, (x_ + 2) % 3
                eng.tensor_tensor(out=yax[:, x_, B_], in0=n_u[:, i1, :],
                                  in1=x_u[:, i2, B_], op=OP.mult)
                eng.tensor_tensor(out=tmp3[:, x_, B_], in0=n_u[:, i2, :],
                                  in1=x_u[:, i1, B_], op=OP.mult)
            eng.tensor_tensor(out=yax[:, :, B_], in0=yax[:, :, B_],
                              in1=tmp3[:, :, B_], op=OP.subtract)

            eng.tensor_copy(out=outa_sb[:, :, b], in_=a_u)

        # aligned-geo half-batch buffers [p, s, q_lo, c32]; pads zeroed once
        alq_A = al_pool.tile([128, S, 4, 32], F16, name="alq_A")
        alq_B = al_pool.tile([128, S, 4, 32], F16, name="alq_B")
        nc.vector.memset(alq_A[:], 0.0)
        nc.gpsimd.memset(alq_B[:], 0.0)

        def emit_rotation(eng, b, q_hi, alq):
            """aligned geo (c = 3i+t) for half-batch q_hi of batch b.
            i emitted in order [2, 0, 1]: n_u is ready before x_u/yax."""
            qs = slice(b * NQ + 4 * q_hi, b * NQ + 4 * q_hi + 4)
            qsl = slice(4 * q_hi, 4 * q_hi + 4)
            rrows = [x_u, yax, u2[:, :, 0, :]]
            svs = [nrmx, zero, dot]
            for i in (2, 0, 1):
                out3 = alq[:, :, :, 3 * i:3 * i + 3].transpose([0, 2, 1, 3])
                for j in range(3):
                    rb = rrows[i][:, j, qs].unsqueeze(2).unsqueeze(3) \
                        .broadcast_to([128, 4, S, 3])
                    src = geo_pt[b][:, qsl, :, j, :]
                    if j == 0:
                        eng.tensor_tensor(out=out3, in0=src, in1=rb, op=OP.mult)
                    else:
                        t = tmp_pool.tile([128, 4, S, 3], F32, tag="rtmp")
                        eng.tensor_tensor(out=t[:], in0=src, in1=rb, op=OP.mult)
                        eng.tensor_tensor(out=out3, in0=out3, in1=t[:], op=OP.add)
                dir_row = alq[:, :, :, 3 * i + 2].transpose([0, 2, 1])
                sv_b = svs[i][:, qs].unsqueeze(2).broadcast_to([128, 4, S])
                eng.tensor_tensor(out=dir_row, in0=sv_b, in1=dir_row,
                                  op=OP.subtract)

        def emit_gt_write(b, q_hi, alq):
            with tc.high_priority():
                nc.gpsimd.dma_start(out=geoT_d[b, q_hi], in_=alq[:])

        def emit_gt_read(b, q_hi):
            gt = gt_pool.tile([128, 4096], F16, tag="gt")
            with tc.high_priority():
                nc.sync.dma_start(
                    out=gt[:],
                    in_=geoT_d[b, q_hi].rearrange("p s ql c -> (p s) (ql c)"),
                    transpose=True)
            gts[(b, q_hi)] = gt

        # ================= startup emission =================
        emit_geo_load(0)
        emit_geo_load(1)
        emit_feats_load(0, 0, 2)

        emit_azi(0)
        emit_rphase(0, nc.vector)
        emit_rotation(nc.vector, 0, 0, alq_A)
        emit_gt_write(0, 0, alq_A)
        emit_gt_read(0, 0)

        emit_azi(1)
        emit_rphase(1, nc.gpsimd)
        nc.sync.dma_start(out=outa_d[:, :, :, :], in_=outa_sb[:])

        emit_feats_load(0, 2, 2)
        emit_feats_load(0, 4, 4)
        emit_rotation(nc.gpsimd, 0, 1, alq_B)
        emit_gt_write(0, 1, alq_B)
        emit_gt_read(0, 1)

        emit_feats_load(1, 0, 4)
        emit_feats_load(1, 4, 4)
        for q_hi, alq in ((0, alq_A), (1, alq_B)):
            emit_rotation(nc.gpsimd, 1, q_hi, alq)
            emit_gt_write(1, q_hi, alq)
            emit_gt_read(1, q_hi)

        pooled = {}
        pooled_raw = {}
        for b in range(BL):
            pooled[b] = pb_pool.tile([128, P], F16, name=f"pooled_{b}")
            pooled_raw[b] = pb_pool.tile([128, P], F16, name=f"pooledr_{b}")

        # PE warmup gated on rot-b0h0 output
        warm_ps = ps1_pool.tile([128, 1024], F32, tag="h1ps")
        warm_rhs = alq_A[:].rearrange("p s q c -> p (s q c)")
        for _ in range(8):
            nc.tensor.matmul(out=warm_ps[:, 0:512], lhsT=w2T[:],
                             rhs=warm_rhs[:, 0:512], start=True, stop=True)

        # ================= chunk loop =================
        chunks = [(b, k) for b in range(BL) for k in range(NCH)]

        def emit_mm1(idx):
            b, k = chunks[idx]
            ql, j = divmod(k, 4)
            q_hi, q_lo = divmod(ql, 4)
            base = j * 1024
            ft = featsT[b]
            gt = gts[(b, q_hi)]
            h1ps = ps1_pool.tile([128, 1024], F32, tag="h1ps")
            gsl = slice(32 * q_lo, 32 * q_lo + 32)
            mfs, mgs = [], []
            for h in (0, 1):
                cs = slice(base + 512 * h, base + 512 * h + 512)
                o = h1ps[:, 512 * h:512 * h + 512]
                mfs.append(nc.tensor.matmul(out=o, lhsT=w1f[:],
                                            rhs=ft[:, ql, cs],
                                            start=True, stop=False))
            for h in (0, 1):
                cs = slice(base + 512 * h, base + 512 * h + 512)
                o = h1ps[:, 512 * h:512 * h + 512]
                mgs.append(nc.tensor.matmul(out=o, lhsT=w1g32[gsl, :],
                                            rhs=gt[gsl, cs],
                                            start=False, stop=True,
                                            tile_position=(32 * q_lo, 0)))
            if SKIP_DUP_LDW:
                mfs[1].ins.ldweights = False
                mgs[1].ins.ldweights = False
            return h1ps

        h1ps_cur = emit_mm1(0)
        for idx, (b, k) in enumerate(chunks):
            h1sb = h1_pool.tile([128, 1024], F16, tag="h1sb")
            nc.scalar.activation(h1sb[:], h1ps_cur[:], AF.Relu, bias=b1t[:, 0:1])
            if idx + 1 < len(chunks):
                h1ps_cur = emit_mm1(idx + 1)
            h2ps = ps2_pool.tile([128, 1024], F32, tag="h2ps")
            nc.tensor.matmul(out=h2ps[:, 0:512], lhsT=w2T[:],
                             rhs=h1sb[:, 0:512], start=True, stop=True)
            m1 = nc.tensor.matmul(out=h2ps[:, 512:1024], lhsT=w2T[:],
                                  rhs=h1sb[:, 512:1024], start=True, stop=True)
            if SKIP_DUP_LDW:
                m1.ins.ldweights = False
            po = k * 32
            nc.vector.reduce_max(
                out=pooled_raw[b][:, po:po + 32],
                in_=h2ps[:].rearrange("m (p s) -> m p s", s=S),
                axis=AX.X)
            if k % 8 == 7:
                seg = slice(po + 32 - 256, po + 32)
                nc.scalar.activation(pooled[b][:, seg], pooled_raw[b][:, seg],
                                     AF.Relu, bias=b2t[:, 0:1])
                nc.sync.dma_start(out=outp_d[b, :, seg], in_=pooled[b][:, seg])

    nc.finalize()
    return nc


_CACHE = {}


def _get_program():
    if "nc" not in _CACHE:
        _CACHE["nc"] = build_program()
    return _CACHE["nc"]


def make_in_maps(input, normal, w1, b1, w2, b2):
    input = np.asarray(input, dtype=np.float32)
    normal = np.asarray(normal, dtype=np.float32)
    w1 = np.asarray(w1, dtype=np.float32)
    b1 = np.asarray(b1, dtype=np.float32)
    w2 = np.asarray(w2, dtype=np.float32)
    b2 = np.asarray(b2, dtype=np.float32)

    w1f = np.ascontiguousarray(w1[:, 3:67].T.astype(np.float16))
    w1g9 = w1[:, GEO_W1_COLS].T.astype(np.float16)   # [9, 128]
    w1g = np.zeros((128, 128), np.float16)
    for kblk in range(4):
        w1g[32 * kblk:32 * kblk + 9] = w1g9
    w2T = np.ascontiguousarray(w2.T.astype(np.float16))
    b1c = np.ascontiguousarray(b1.reshape(128, 1))
    b2c = np.ascontiguousarray(b2.reshape(128, 1))

    in_maps = []
    for core in range(NCORES):
        b0 = core * BL
        inp = input[b0:b0 + BL]
        f = inp[:, 12:76].astype(np.float16)
        feats = np.ascontiguousarray(f.reshape(BL, 64, NQ, 4096))
        g = inp[:, 3:12].astype(np.float16)
        # channels 3:12 = (t, j); want [p, q, s, j, t]
        g = g.reshape(BL, 3, 3, NQ, 128, S).transpose(0, 4, 3, 5, 2, 1)
        geo = np.ascontiguousarray(g)
        normp = np.ascontiguousarray(
            normal[b0:b0 + BL].reshape(BL, NQ, 128, 3).transpose(2, 3, 0, 1))
        in_maps.append({
            "feats": feats, "geo": geo, "normp": normp,
            "w1f": w1f, "w1g": w1g, "w2T": w2T, "b1c": b1c, "b2c": b2c,
            "geot": np.zeros((BL, 2, 128, S, 4, 32), np.float16),
        })
    return in_maps


def assemble_output(results):
    outs = []
    for r in results:
        outp = r["outp"].astype(np.float32)   # (BL,128,P)
        outa = r["outa"]                      # (128,3,BL,NQ)
        azi = outa.transpose(2, 1, 3, 0).reshape(BL, 3, P)
        outs.append(np.concatenate([azi, outp], axis=1))
    return np.concatenate(outs, axis=0)


def kernel(input, normal, w1, b1, w2, b2, _trace=False):
    nc = _get_program()
    in_maps = make_in_maps(input, normal, w1, b1, w2, b2)
    res = run_bass_kernel_spmd(nc, in_maps, core_ids=list(range(NCORES)), trace=_trace)
    out = assemble_output(res.results)
    if _trace:
        return out, res
    return out
